# revision 17
# baseline (speedup 1.0000x reference)
"""Differential entropy regularization (retrieval_knn) on 8 Trainium2 cores.

loss = -mean_i log( mean_{k in top5} ||xn_i - xn_j(k)|| + eps ),  xn = row-normalized x.

Key algebra used by the kernel:
  * For unit rows, distance(i,j) = sqrt(2 - 2 * dot(xn_i, xn_j)), so only the
    top-5 dot VALUES per row are needed (no index gathers).
  * The self-dot (== 1) is always the strict row max, so taking the top-8
    values per row and dropping element 0 replaces diagonal masking.
  * Scaling row i of the similarity matrix by a positive constant r_i does not
    change which columns are its top-5.  So the stationary (lhsT) operand can
    stay UN-normalized (raw rows); only the moving operand (all columns) is
    normalized, and the row scale is divided back out inside the final
    sqrt(2 - 2v) evaluation via a per-partition activation scale.

Sharding: rows are split 1024 per core; every core receives the full x^T
(moving operand) so no device collectives are needed.  Per core:
  gram = raw_rows_block^T @ xn  via fp32r matmuls (N=512 -> full PE rate),
  per 512-col tile: DVE max8 straight out of PSUM -> candidate top-8s,
  merge -> row top-8, ACT computes sqrt / log partials, host averages.
"""

import numpy as np


def _ensure_path():
    try:
        import concourse.bass  # noqa: F401
    except ImportError:
        import sys

        for p in ("/opt/trn_rl_repo", "/root/.axon_site/_ro/trn_rl_repo"):
            if p not in sys.path:
                sys.path.insert(0, p)
        import concourse.bass  # noqa: F401


N = 8192  # total rows
D = 512  # feature dim
NCORES = 8
RPC = N // NCORES  # rows per core (1024)
P = 128  # partitions
KC = D // P  # contraction chunks (4)
CW = 512  # gram col tile width
CT = N // CW  # col tiles (16)
RT = RPC // P  # row tiles per core (8)
H = 4096  # xt column half width (per SBUF tile)
NH = N // H  # halves (2)
EPS = 1e-8

_NC_CACHE = {}


def _build_nc():
    """Build the (identical-per-core) Bass program once."""
    import concourse.bass as bass  # noqa: F401
    import concourse.tile as tile
    from concourse import bacc, mybir
    from contextlib import ExitStack

    f32 = mybir.dt.float32
    f32r = mybir.dt.float32r
    AF = mybir.ActivationFunctionType

    nc = bacc.Bacc(trn_type="TRN2", target_bir_lowering=False, debug=False)

    xt_d = nc.dram_tensor("xt", [D, N], f32r, kind="ExternalInput")
    xtr_d = nc.dram_tensor("xtr", [D, RPC], f32r, kind="ExternalInput")
    xr_d = nc.dram_tensor("xr", [RPC, D], f32, kind="ExternalInput")
    ones_d = nc.dram_tensor("onesvec", [P, 1], f32r, kind="ExternalInput")
    out_d = nc.dram_tensor("out", [P, RT], f32, kind="ExternalOutput")
    rn_d = nc.dram_tensor("rn_scratch", [1, N], f32)

    with ExitStack() as ctx:
        tc = ctx.enter_context(tile.TileContext(nc))
        res = ctx.enter_context(tc.tile_pool(name="res", bufs=1))

        # ---- resident loads -------------------------------------------------
        XT = {}
        for k in range(KC):
            for h in range(NH):
                t = res.tile([P, H], f32r, name=f"xt_{k}_{h}")
                nc.sync.dma_start(t, xt_d.ap()[k * P : (k + 1) * P, h * H : (h + 1) * H])
                XT[k, h] = t
        XTR = []
        for k in range(KC):
            t = res.tile([P, RPC], f32r, name=f"xtr_{k}")
            nc.sync.dma_start(t, xtr_d.ap()[k * P : (k + 1) * P, :])
            XTR.append(t)
        ones = res.tile([P, 1], f32r, name="ones")
        nc.sync.dma_start(ones, ones_d.ap())
        btwo = res.tile([P, 1], f32, name="btwo")
        nc.vector.memset(btwo, 2.0)
        beps = res.tile([P, 1], f32, name="beps")
        nc.vector.memset(beps, EPS)

        # ---- row norms for this core's rows -> RM2 = -2 / ||x_row|| --------
        SSR = res.tile([P, RT], f32, name="ssr")
        SQR = res.tile([P, RT], f32, name="sqr")
        RINV = res.tile([P, RT], f32, name="rinv")
        RM2 = res.tile([P, RT], f32, name="rm2")
        scratch = ctx.enter_context(tc.tile_pool(name="scratch", bufs=3))
        for rt in range(RT):
            xrt = scratch.tile([P, D], f32, tag="xrt")
            nc.sync.dma_start(xrt, xr_d.ap()[rt * P : (rt + 1) * P, :])
            dummy = scratch.tile([P, D], f32, tag="sq")
            nc.scalar.activation(
                dummy, xrt, AF.Square, accum_out=SSR[:, rt : rt + 1]
            )
        nc.scalar.activation(SQR, SSR, AF.Sqrt)
        nc.vector.reciprocal(RINV, SQR)
        nc.scalar.mul(RM2, RINV, -2.0)

        # ---- column norms of ALL columns -> rn_d ---------------------------
        with tc.tile_pool(name="psn", bufs=2, space="PSUM") as psn:
            for ct in range(CT):
                h, hoff = divmod(ct * CW, H)
                ps = psn.tile([1, CW], f32, tag="ssps")
                for k in range(KC):
                    sq = scratch.tile([P, CW], f32r, tag="sq")
                    nc.scalar.activation(sq, XT[k, h][:, hoff : hoff + CW], AF.Square)
                    nc.tensor.matmul(
                        ps,
                        lhsT=ones,
                        rhs=sq,
                        start=(k == 0),
                        stop=(k == KC - 1),
                    )
                ssb = scratch.tile([1, CW], f32, tag="ssb")
                nc.scalar.activation(ssb, ps, AF.Sqrt)
                rnb = scratch.tile([1, CW], f32, tag="rnb")
                nc.vector.reciprocal(rnb, ssb)
                nc.sync.dma_start(rn_d.ap()[:, ct * CW : (ct + 1) * CW], rnb)

        # ---- broadcast rn across partitions, normalize columns in place ---
        for h in range(NH):
            RNh = res.tile([P, H], f32, name=f"rn_{h}")
            nc.sync.dma_start(
                RNh, rn_d.ap()[:, h * H : (h + 1) * H].to_broadcast((P, H))
            )
            for k in range(KC):
                nc.vector.tensor_mul(XT[k, h], XT[k, h], RNh)

        # ---- gram blocks + per-row top-8 + loss partials -------------------
        psg = ctx.enter_context(tc.tile_pool(name="psg", bufs=6, space="PSUM"))
        gp = ctx.enter_context(tc.tile_pool(name="gp", bufs=2))
        OUT = res.tile([P, RT], f32, name="outv")
        for rt in range(RT):
            cand = gp.tile([P, 8 * CT], f32, tag="cand")
            for ct in range(CT):
                h, hoff = divmod(ct * CW, H)
                ps = psg.tile([P, CW], f32, tag="gram")
                for k in range(KC):
                    nc.tensor.matmul(
                        ps,
                        lhsT=XTR[k][:, rt * P : (rt + 1) * P],
                        rhs=XT[k, h][:, hoff : hoff + CW],
                        start=(k == 0),
                        stop=(k == KC - 1),
                    )
                nc.vector.max(out=cand[:, ct * 8 : (ct + 1) * 8], in_=ps)
            top8 = gp.tile([P, 8], f32, tag="top8")
            nc.vector.max(out=top8, in_=cand)
            # f = sqrt(2 - 2 * v / r_i)   (values in top8 are r_i-scaled)
            f5 = gp.tile([P, 5], f32, tag="f5")
            rho = gp.tile([P, 1], f32, tag="rho")
            nc.scalar.activation(
                f5,
                top8[:, 1:6],
                AF.Sqrt,
                bias=btwo[:, 0:1],
                scale=RM2[:, rt : rt + 1],
                accum_out=rho,
            )
            # out = ln(rho/5 + eps)
            nc.scalar.activation(
                OUT[:, rt : rt + 1], rho, AF.Ln, bias=beps[:, 0:1], scale=0.2
            )
        nc.sync.dma_start(out_d.ap(), OUT)

    nc.compile()
    return nc


def get_nc():
    if "nc" not in _NC_CACHE:
        _ensure_path()
        _NC_CACHE["nc"] = _build_nc()
    return _NC_CACHE["nc"]


def make_in_maps(x):
    x = np.ascontiguousarray(np.asarray(x, dtype=np.float32))
    assert x.shape == (N, D), x.shape
    xt = np.ascontiguousarray(x.T)
    in_maps = []
    for c in range(NCORES):
        in_maps.append(
            {
                "xt": xt,
                "xtr": np.ascontiguousarray(xt[:, c * RPC : (c + 1) * RPC]),
                "xr": np.ascontiguousarray(x[c * RPC : (c + 1) * RPC, :]),
                "onesvec": np.ones((P, 1), dtype=np.float32),
            }
        )
    return in_maps


def combine(results):
    """results: list (per core) of {"out": [P, RT]} -> scalar loss."""
    vals = []
    for c in range(NCORES):
        o = np.asarray(results[c]["out"])  # [P, RT]; row = c*RPC + rt*P + p
        vals.append(o.T.reshape(-1))
    allv = np.concatenate(vals)
    return np.array(-np.mean(allv), dtype=np.float32)


def run(x, **spmd_kwargs):
    _ensure_path()
    from concourse.bass_utils import run_bass_kernel_spmd

    nc = get_nc()
    res = run_bass_kernel_spmd(nc, make_in_maps(x), list(range(NCORES)), **spmd_kwargs)
    return combine(res.results), res


def kernel(x):
    loss, _ = run(x)
    return loss


# revision 22
# speedup vs baseline: 1.0163x; 1.0163x over previous
"""Differential entropy regularization (retrieval_knn) on 8 Trainium2 cores.

loss = -mean_i log( mean_{k in top5} ||xn_i - xn_j(k)|| + eps ),  xn = row-normalized x.

Key algebra used by the kernel:
  * For unit rows, distance(i,j) = sqrt(2 - 2 * dot(xn_i, xn_j)), so only the
    top-5 dot VALUES per row are needed (no index gathers).
  * The self-dot (== 1) is always the strict row max, so taking the top-8
    values per row via the DVE max8 instruction and dropping element 0
    replaces diagonal masking.
  * Scaling row i of the similarity matrix by a positive constant r_i does not
    change which columns are its top-5.  So the stationary (lhsT) operand
    stays UN-normalized (raw rows); only the moving operand (all columns) is
    normalized, and the row scale is divided back out inside the final
    sqrt(2 - 2v) evaluation via a per-partition activation scale.

Sharding: rows are split 1024 per core; every core receives the full x^T
(moving operand, fp32r) so the gram needs no communication.  Reciprocal row
norms are computed per-core from the natural-layout row slice (ACT square
with accumulate), AllGathered (8 x 4KB) to give every core all 8192 column
norms, partition-broadcast by DMA, and multiplied into x^T (DVE for the
early column quarters on the critical path, GpSimd for the late ones).
Per core: 512 fp32r matmuls (N=512, full PE rate) -> [128, 2048] PSUM tiles
-> DVE max8 straight out of PSUM -> merge -> ACT sqrt/log partials -> host
mean.
"""

import numpy as np


def _ensure_path():
    try:
        import concourse.bass  # noqa: F401
    except ImportError:
        import sys

        for p in ("/opt/trn_rl_repo", "/root/.axon_site/_ro/trn_rl_repo"):
            if p not in sys.path:
                sys.path.insert(0, p)
        import concourse.bass  # noqa: F401


N = 8192  # total rows
D = 512  # feature dim
NCORES = 8
RPC = N // NCORES  # rows per core (1024)
P = 128  # partitions
KC = D // P  # contraction chunks (4)
CW = 512  # matmul moving free dim
Q = 2048  # PSUM tile width / column quarter width per half... (gram tile)
NQ = N // Q  # 4 gram quarters of the full column range
RT = RPC // P  # row tiles per core (8)
EPS = 1e-8

_NC_CACHE = {}


def _build_nc():
    """Build the (identical-per-core) Bass program once."""
    import concourse.bass as bass  # noqa: F401
    import concourse.tile as tile
    from concourse import bacc, mybir
    from contextlib import ExitStack

    f32 = mybir.dt.float32
    f32r = mybir.dt.float32r
    AF = mybir.ActivationFunctionType

    nc = bacc.Bacc(trn_type="TRN2", target_bir_lowering=False, debug=False)

    xt_d = nc.dram_tensor("xt", [D, N], f32r, kind="ExternalInput")
    xtr_d = nc.dram_tensor("xtr", [D, RPC], f32r, kind="ExternalInput")
    xr_d = nc.dram_tensor("xr", [RPC, D], f32, kind="ExternalInput")
    out_d = nc.dram_tensor("out", [P, RT], f32, kind="ExternalOutput")
    rn_own_d = nc.dram_tensor("rn_own", [1, RPC], f32)
    rn_all_d = nc.dram_tensor("rn_all", [NCORES, RPC], f32)
    rn_flat_d = nc.dram_tensor("rn_flat", [1, N], f32)

    with ExitStack() as ctx:
        tc = ctx.enter_context(tile.TileContext(nc))
        res = ctx.enter_context(tc.tile_pool(name="res", bufs=1))

        # ---- resident loads (xt split per (k-chunk, column quarter)) -------
        XT = {}
        for k in range(KC):
            for q in range(NQ):
                t = res.tile([P, Q], f32r, name=f"xt_{k}_{q}")
                nc.sync.dma_start(t, xt_d.ap()[k * P : (k + 1) * P, q * Q : (q + 1) * Q])
                XT[k, q] = t
        XTR = []
        for k in range(KC):
            t = res.tile([P, RPC], f32r, name=f"xtr_{k}")
            nc.sync.dma_start(t, xtr_d.ap()[k * P : (k + 1) * P, :])
            XTR.append(t)

        # ---- own-row norms: SSR[p, rt] = sum_d x[row]^2, row = rt*P+p ------
        SSR = res.tile([P, RT], f32, name="ssr")
        SROOT = res.tile([P, RT], f32, name="sroot")
        RINV = res.tile([P, RT], f32, name="rinv")
        RM2 = res.tile([P, RT], f32, name="rm2")
        btwo = res.tile([P, 1], f32, name="btwo")
        nc.vector.memset(btwo, 2.0)
        beps = res.tile([P, 1], f32, name="beps")
        nc.vector.memset(beps, EPS)
        scratch = ctx.enter_context(tc.tile_pool(name="scratch", bufs=3))
        for rt in range(RT):
            xrt = scratch.tile([P, D], f32, tag="xrt")
            nc.sync.dma_start(xrt, xr_d.ap()[rt * P : (rt + 1) * P, :])
            dummy = scratch.tile([P, D], f32, tag="sq")
            nc.scalar.activation(dummy, xrt, AF.Square, accum_out=SSR[:, rt : rt + 1])
        nc.scalar.activation(SROOT, SSR, AF.Sqrt)
        nc.vector.reciprocal(RINV, SROOT)
        nc.vector.tensor_scalar_mul(RM2, RINV, -2.0)

        # ---- share reciprocal norms: own [1024] -> all [8192] --------------
        # rn_own[rt*P + p] = RINV[p, rt]
        nc.sync.dma_start(
            rn_own_d.ap().rearrange("o (t p) -> o p t", p=P), RINV
        )
        nc.gpsimd.collective_compute(
            "AllGather",
            mybir.AluOpType.bypass,
            replica_groups=[list(range(NCORES))],
            ins=[rn_own_d.ap()],
            outs=[rn_all_d.ap()],
        )
        nc.sync.dma_start(rn_flat_d.ap(), rn_all_d.ap().rearrange("r j -> (r j)"))

        # ---- broadcast rn across partitions, normalize columns in place ---
        # DVE handles the first quarters (critical path: gram q0 starts after
        # TT q0); GpSimd handles the tail quarters in the shadow of the gram.
        for q in range(NQ):
            RNq = res.tile([P, Q], f32, name=f"rn_{q}")
            nc.sync.dma_start(
                RNq, rn_flat_d.ap()[:, q * Q : (q + 1) * Q].to_broadcast((P, Q))
            )
            eng = nc.vector if q < 2 else nc.gpsimd
            for k in range(KC):
                eng.tensor_mul(XT[k, q], XT[k, q], RNq)

        # ---- gram quarters + per-row top-8 + loss partials -----------------
        psg = ctx.enter_context(tc.tile_pool(name="psg", bufs=2, space="PSUM"))
        gp = ctx.enter_context(tc.tile_pool(name="gp", bufs=1))
        CAND = [
            gp.tile([P, 8 * NQ], f32, tag=f"cand{rt}", name=f"cand{rt}")
            for rt in range(RT)
        ]
        RHO = res.tile([P, RT], f32, name="rho")
        OUT = res.tile([P, RT], f32, name="outv")
        for q in range(NQ):
            for rt in range(RT):
                ps = psg.tile([P, Q], f32, tag="gram")
                for sub in range(Q // CW):
                    for k in range(KC):
                        nc.tensor.matmul(
                            ps[:, sub * CW : (sub + 1) * CW],
                            lhsT=XTR[k][:, rt * P : (rt + 1) * P],
                            rhs=XT[k, q][:, sub * CW : (sub + 1) * CW],
                            start=(k == 0),
                            stop=(k == KC - 1),
                        )
                nc.vector.max(out=CAND[rt][:, q * 8 : (q + 1) * 8], in_=ps)
        f5p = ctx.enter_context(tc.tile_pool(name="f5p", bufs=2))
        for rt in range(RT):
            top8 = f5p.tile([P, 8], f32, tag="top8")
            nc.vector.max(out=top8, in_=CAND[rt])
            # f = sqrt(2 - 2 * v / r_i)   (values in top8 are r_i-scaled)
            f5 = f5p.tile([P, 5], f32, tag="f5")
            nc.scalar.activation(
                f5,
                top8[:, 1:6],
                AF.Sqrt,
                bias=btwo[:, 0:1],
                scale=RM2[:, rt : rt + 1],
                accum_out=RHO[:, rt : rt + 1],
            )
        # out = ln(rho/5 + eps), batched over all row tiles
        nc.scalar.activation(OUT, RHO, AF.Ln, bias=beps[:, 0:1], scale=0.2)
        nc.sync.dma_start(out_d.ap(), OUT)

    nc.compile()
    return nc


def get_nc():
    if "nc" not in _NC_CACHE:
        _ensure_path()
        _NC_CACHE["nc"] = _build_nc()
    return _NC_CACHE["nc"]


def make_in_maps(x):
    x = np.ascontiguousarray(np.asarray(x, dtype=np.float32))
    assert x.shape == (N, D), x.shape
    xt = np.ascontiguousarray(x.T)
    in_maps = []
    for c in range(NCORES):
        in_maps.append(
            {
                "xt": xt,
                "xtr": np.ascontiguousarray(xt[:, c * RPC : (c + 1) * RPC]),
                "xr": np.ascontiguousarray(x[c * RPC : (c + 1) * RPC, :]),
            }
        )
    return in_maps


def combine(results):
    """results: list (per core) of {"out": [P, RT]} -> scalar loss."""
    vals = []
    for c in range(NCORES):
        o = np.asarray(results[c]["out"])  # [P, RT]; row = c*RPC + rt*P + p
        vals.append(o.T.reshape(-1))
    allv = np.concatenate(vals)
    return np.array(-np.mean(allv), dtype=np.float32)


def run(x, **spmd_kwargs):
    _ensure_path()
    from concourse.bass_utils import run_bass_kernel_spmd

    nc = get_nc()
    res = run_bass_kernel_spmd(nc, make_in_maps(x), list(range(NCORES)), **spmd_kwargs)
    return combine(res.results), res


def kernel(x):
    loss, _ = run(x)
    return loss
